# revision 10
# baseline (speedup 1.0000x reference)
"""BGK collision operator kernel for 8 Trainium2 NeuronCores.

omega[n,q] = (f_eq[n,q] - f[n,q]) / tau[n]

Key algebraic simplifications vs the reference:
  * The Newton solve has a closed form: the quadrature grid is uniform
    (xi_q ~= q*D), so the equilibrium weights are geometric with ratio
    r = v/(v+D), giving lam = ln(v/(v+D))/D and S0 = (v+D)/D.  The
    reference's 100 fixed Newton iterations converge to this same root
    (verified to ~3e-7 relative).
  * f_eq/tau is computed as a single exponential:
      f_eq[n,q]/tau[n] = exp(q*lam' + bias[n]),  lam' = ln(v)-ln(v+D)
      bias = ln(sum_q f) - ln(v+D) - (z+b4) + ln(D/64)
    where z+b4 is the MLP output (tau = exp(z+b4)).
  * r <= 1/(1+D) = 0.474, so exp terms for q >= 32 are < 5e-11 of scale
    and are exactly 0 at fp32 downstream: omega[:, 32:] = -f/tau.
"""

import numpy as np
from contextlib import ExitStack

import concourse.bass as bass
import concourse.tile as tile
from concourse import bacc, mybir
from concourse import bass_utils

# ---------------------------------------------------------------- constants
N_FULL = 500000
Q = 64
QK = 32                  # quadrature points with non-negligible f_eq
NCORES = 8
TILE_ROWS = 4096         # rows per macro-tile (32 blocks of 128)
TILES_PER_CORE = 16
R_CORE = TILE_ROWS * TILES_PER_CORE          # 65536
N_PAD = R_CORE * NCORES                      # 524288

# exact fp32 bits of jnp.linspace(0, 70, 64) (differs from np.linspace in ulps)
XI = np.array([
    0x00000000, 0x3f8e38e4, 0x400e38e4, 0x40555556, 0x408e38e4, 0x40b1c71d,
    0x40d55556, 0x40f8e38f, 0x410e38e4, 0x41200000, 0x4131c71d, 0x41438e3a,
    0x41555556, 0x41671c72, 0x4178e38f, 0x41855556, 0x418e38e4, 0x41971c72,
    0x41a00000, 0x41a8e38f, 0x41b1c71d, 0x41baaaab, 0x41c38e3a, 0x41cc71c8,
    0x41d55556, 0x41de38e4, 0x41e71c72, 0x41f00001, 0x41f8e38f, 0x4200e38f,
    0x42055556, 0x4209c71d, 0x420e38e4, 0x4212aaab, 0x42171c72, 0x421b8e39,
    0x42200000, 0x422471c8, 0x4228e38f, 0x422d5556, 0x4231c71d, 0x423638e4,
    0x423aaaab, 0x423f1c72, 0x42438e3a, 0x42480001, 0x424c71c8, 0x4250e38f,
    0x42555556, 0x4259c71d, 0x425e38e4, 0x4262aaab, 0x42671c72, 0x426b8e3a,
    0x42700001, 0x427471c8, 0x4278e38f, 0x427d5556, 0x4280e38f, 0x42831c72,
    0x42855556, 0x42878e39, 0x4289c71d, 0x428c0000,
], dtype=np.uint32).view(np.float32)
DELTA = np.float64(70.0) / np.float64(63.0)
D32 = np.float32(DELTA)

F32 = mybir.dt.float32
AF = mybir.ActivationFunctionType
ALU = mybir.AluOpType


def _consts_array(Ws, bs):
    """Host-side consts tile [128, NC]: xiD, replicated biases, block-diag
    packed (pre-transposed) weights, and scalar bias columns."""
    W0, W1, W2, W3, W4 = Ws
    b0, b1, b2, b3, b4 = bs
    cols = {}
    c = np.zeros((128, 560), dtype=np.float32)
    # xiD[q] = xi_q / D for q < 32, replicated on all partitions
    xiD = (XI[:QK].astype(np.float64) / DELTA).astype(np.float32)
    c[:, 0:QK] = xiD[None, :]
    cols["xiD"] = (0, QK)
    # replicated hidden biases [b;b]
    for i, b in enumerate([b0, b1, b2, b3]):
        c[0:64, QK + i] = b
        c[64:128, QK + i] = b
        cols[f"b{i}r"] = (QK + i, QK + i + 1)
    o = QK + 4
    # lhsT0 [3, 64] = W0.T (W0 is [64, 3]); placed per col-group at matmul
    c[0:3, o:o + 64] = W0.T
    cols["lhsT0"] = (o, o + 64)
    o += 64
    for i, W in enumerate([W1, W2, W3]):
        c[0:64, o:o + 64] = W.T
        c[64:128, o + 64:o + 128] = W.T
        cols[f"lhsT{i + 1}"] = (o, o + 128)
        o += 128
    # lhsT4 [128, 2]
    c[0:64, o] = W4[0, :]
    c[64:128, o + 1] = W4[0, :]
    cols["lhsT4"] = (o, o + 2)
    o += 2
    # scalar columns (replicated across partitions)
    nb4 = np.float32(-float(b4[0]))
    CB = np.float32(np.log(DELTA / 64.0))
    c[:, o] = nb4
    cols["nb4"] = (o, o + 1)
    c[:, o + 1] = CB
    cols["CB"] = (o + 1, o + 2)
    o += 2
    return c[:, :o].copy(), cols


def build_nc(tiles_per_core=TILES_PER_CORE, nc_cols=None):
    rows = TILE_ROWS * tiles_per_core
    nc = bacc.Bacc("TRN2", target_bir_lowering=False, debug=False,
                   num_devices=NCORES)
    f_d = nc.dram_tensor("f", [rows, Q], F32, kind="ExternalInput").ap()
    xm_d = nc.dram_tensor("xm", [2, rows], F32, kind="ExternalInput").ap()
    xp_d = nc.dram_tensor("xp", [1, rows], F32, kind="ExternalInput").ap()
    cst_d = nc.dram_tensor("consts", [128, nc_cols], F32,
                           kind="ExternalInput").ap()
    out_d = nc.dram_tensor("out", [rows, Q], F32, kind="ExternalOutput").ap()

    with tile.TileContext(nc) as tc, ExitStack() as ctx:
        cpool = ctx.enter_context(tc.tile_pool(name="consts", bufs=1))
        main = ctx.enter_context(tc.tile_pool(name="main", bufs=2))
        hpool = ctx.enter_context(tc.tile_pool(name="h", bufs=3))
        psA = ctx.enter_context(tc.tile_pool(name="psA", bufs=1, space="PSUM"))
        psB = ctx.enter_context(tc.tile_pool(name="psB", bufs=1, space="PSUM"))
        dpool = ctx.enter_context(tc.tile_pool(name="dram", bufs=2,
                                               space="DRAM"))

        cst = cpool.tile([128, nc_cols], F32)
        nc.sync.dma_start(cst[:], cst_d)

        def cc(name):
            a, b = build_nc.cols[name]
            return cst[:, a:b]

        for i in range(tiles_per_core):
            # row mapping within a macro-tile: row = base + 32*p + j
            # (partition p holds 32 consecutive rows) -> every DMA below is
            # contiguous.
            base = i * TILE_ROWS
            # ---- loads
            f_t = main.tile([128, 32 * Q], F32, tag="f_t")
            nc.sync.dma_start(
                f_t[:],
                f_d[base:base + TILE_ROWS, :].rearrange(
                    "(p j) q -> p (j q)", p=128))
            pk = main.tile([128, 96], F32, tag="pk")
            nc.sync.dma_start(
                pk[:, 0:32],
                xm_d[0:1, base:base + TILE_ROWS].rearrange(
                    "d (p j) -> p (d j)", p=128))
            x_fm = main.tile([3, TILE_ROWS], F32, tag="x_fm")
            nc.sync.dma_start(x_fm[0:2, :], xm_d[:, base:base + TILE_ROWS])
            nc.sync.dma_start(x_fm[2:3, :], xp_d[:, base:base + TILE_ROWS])

            # ---- MLP (feature-major, 2-subtile stacking into 128 partitions)
            # L0 (K=3): two col-group matmuls per pair build the stacked
            # [128, 512] layout directly (half A -> partitions 0:64 via
            # tile_position (0,0), half B -> 64:128 via (0,64)).
            h_ps0 = psA.tile([128, 2048], F32, tag="psA")
            for k in range(4):
                nc.tensor.matmul(h_ps0[0:64, 512 * k:512 * k + 512],
                                 cc("lhsT0")[0:3, :],
                                 x_fm[:, 1024 * k:1024 * k + 512],
                                 start=True, stop=True, tile_position=(0, 0))
                nc.tensor.matmul(h_ps0[64:128, 512 * k:512 * k + 512],
                                 cc("lhsT0")[0:3, :],
                                 x_fm[:, 1024 * k + 512:1024 * k + 1024],
                                 start=True, stop=True, tile_position=(0, 64))
            h = hpool.tile([128, 2048], F32, tag="h")
            nc.scalar.activation(h[:], h_ps0[:], AF.Tanh, bias=cc("b0r"))
            for li, pspool, pstag in ((1, psB, "psB"), (2, psA, "psA"),
                                      (3, psB, "psB")):
                h_ps = pspool.tile([128, 2048], F32, tag=pstag)
                for k in range(4):
                    nc.tensor.matmul(h_ps[:, 512 * k:512 * k + 512],
                                     cc(f"lhsT{li}"),
                                     h[:, 512 * k:512 * k + 512],
                                     start=True, stop=True)
                h = hpool.tile([128, 2048], F32, tag="h")
                nc.scalar.activation(h[:], h_ps[:], AF.Tanh, bias=cc(f"b{li}r"))
            z_ps = psA.tile([128, 2048], F32, tag="psA")
            for k in range(4):
                nc.tensor.matmul(z_ps[0:2, 512 * k:512 * k + 512], cc("lhsT4"),
                                 h[:, 512 * k:512 * k + 512],
                                 start=True, stop=True)
            z_fm = main.tile([2, 2048], F32, tag="z_fm")
            nc.vector.tensor_copy(z_fm[:], z_ps[0:2, :])
            # bounce through DRAM to redistribute [2, 2048] -> [128, 32]:
            # store z into row order (row = 1024k + 512s + c), reload as
            # [128, 32] contiguous (row = 32p + j).
            zd = dpool.tile([1, TILE_ROWS], F32, tag="zd")
            nc.sync.dma_start(
                zd[:].rearrange("a (k s c) -> (a s) k c", k=4, s=2, c=512),
                z_fm[:].rearrange("s (k c) -> s k c", k=4))
            z_cols = main.tile([128, 32], F32, tag="z_cols")
            nc.sync.dma_start(
                z_cols[:], zd[:].rearrange("a (p j) -> (a p) j", p=128))

            # ---- equilibrium scalars
            # negit = -exp(-(z + b4)) = -1/tau
            ng = main.tile([128, 32], F32, tag="ng")
            nc.scalar.activation(ng[:], z_cols[:], AF.Exp, scale=-1.0,
                                 bias=cc("nb4"))
            nc.vector.tensor_scalar_mul(ng[:], ng[:], -1.0)
            # t = v + D
            nc.vector.tensor_scalar_add(pk[:, 32:64], pk[:, 0:32], float(D32))
            # omega[:, :] = f * negit ; acc_j = sum_q (f*negit)
            om = main.tile([128, 32 * Q], F32, tag="om")
            acc = main.tile([128, 32], F32, tag="acc")
            for j in range(32):
                nc.vector.tensor_scalar(
                    om[:, Q * j:Q * j + Q], f_t[:, Q * j:Q * j + Q],
                    ng[:, j:j + 1], 0.0, ALU.mult, ALU.add,
                    accum_out=acc[:, j:j + 1])
            # pk[:,64:96] = -acc = (1/tau) * sum_q f
            nc.vector.tensor_scalar_mul(pk[:, 64:96], acc[:], -1.0)
            lnpk = main.tile([128, 96], F32, tag="lnpk")
            nc.scalar.activation(lnpk[:], pk[:], AF.Ln)
            lamp = main.tile([128, 32], F32, tag="lamp")
            nc.vector.tensor_sub(lamp[:], lnpk[:, 0:32], lnpk[:, 32:64])
            bias0 = main.tile([128, 32], F32, tag="bias0")
            nc.vector.tensor_sub(bias0[:], lnpk[:, 64:96], lnpk[:, 32:64])
            # arg[p, j, q] = xiD[q] * lamp[p,j] + bias0[p,j]
            argt = main.tile([128, 32 * QK], F32, tag="argt")
            for j in range(32):
                nc.vector.tensor_scalar(
                    argt[:, QK * j:QK * j + QK], cc("xiD"),
                    lamp[:, j:j + 1], bias0[:, j:j + 1], ALU.mult, ALU.add)
            wA = main.tile([128, 32 * QK], F32, tag="wA")
            nc.scalar.activation(wA[:], argt[:], AF.Exp, bias=cc("CB"))
            # omega[:, :, :QK] += wA
            omv = om[:].rearrange("p (j q) -> p j q", j=32)
            wv = wA[:].rearrange("p (j q) -> p j q", j=32)
            nc.vector.tensor_add(omv[:, :, 0:QK], omv[:, :, 0:QK], wv)

            nc.sync.dma_start(
                out_d[base:base + TILE_ROWS, :].rearrange(
                    "(p j) q -> p (j q)", p=128),
                om[:])

    nc.finalize()
    return nc


build_nc.cols = None


def _prepare(f_distribution, macro_features, position_embedding, Ws, bs):
    consts, cols = _consts_array(Ws, bs)
    build_nc.cols = cols
    n = f_distribution.shape[0]
    f_pad = np.full((N_PAD, Q), 0.5, dtype=np.float32)
    f_pad[:n] = f_distribution
    xm_pad = np.full((2, N_PAD), 0.5, dtype=np.float32)
    xm_pad[:, :n] = macro_features.T
    xp_pad = np.zeros((1, N_PAD), dtype=np.float32)
    xp_pad[:, :n] = position_embedding.T
    in_maps = []
    for c in range(NCORES):
        sl = slice(c * R_CORE, (c + 1) * R_CORE)
        in_maps.append({
            "f": np.ascontiguousarray(f_pad[sl]),
            "xm": np.ascontiguousarray(xm_pad[:, sl]),
            "xp": np.ascontiguousarray(xp_pad[:, sl]),
            "consts": consts,
        })
    return in_maps, consts.shape[1]


def kernel(f_distribution, macro_features, position_embedding,
           W0, b0, W1, b1, W2, b2, W3, b3, W4, b4):
    f_distribution = np.ascontiguousarray(f_distribution, dtype=np.float32)
    macro_features = np.ascontiguousarray(macro_features, dtype=np.float32)
    position_embedding = np.ascontiguousarray(position_embedding,
                                              dtype=np.float32)
    Ws = [np.asarray(W, dtype=np.float32) for W in (W0, W1, W2, W3, W4)]
    bs = [np.asarray(b, dtype=np.float32) for b in (b0, b1, b2, b3, b4)]
    in_maps, ncols = _prepare(f_distribution, macro_features,
                              position_embedding, Ws, bs)
    nc = build_nc(TILES_PER_CORE, nc_cols=ncols)
    res = bass_utils.run_bass_kernel_spmd(nc, in_maps,
                                          core_ids=list(range(NCORES)))
    out = np.concatenate([res.results[c]["out"] for c in range(NCORES)],
                         axis=0)
    return out[:f_distribution.shape[0]]
